# revision 13
# baseline (speedup 1.0000x reference)
"""GaussianMLP sampling kernel for 8 trn2 NeuronCores (pure data parallel).

reference:
    h      = relu(x @ W_emb + b_emb)        x:[B,128] W_emb:[128,256]
    mean   = h @ W_mean + b_mean            W_mean:[256,128]
    logvar = h @ W_logvar + b_logvar        W_logvar:[256,128]
    z      = mean + exp(0.5*logvar) * eps
    returns (z, mean, logvar)

Fully transposed dataflow: the host pre-transposes x/eps per core to
[d, rows] bf16, the device computes everything in [feature, row] space
(features on partitions), and the host transposes the three bf16
outputs back.  Wins vs. the row-major formulation:
  - no PE transposes and no K=1 bias-seed matmuls: every bias is
    per-partition, folded into ACT/DVE epilogue ops for free
  - all matmuls are bf16 with the small weight as the stationary
    operand and 512-row moving tiles (N=512)
  - all five HBM streams are bf16 (halves traffic vs f32) and fully
    contiguous: 1 MiB DMAs with 8 KiB per-partition runs

Per 512-row compute tile:
  PE : hT0/hT1 = We_c.T @ xT      (2 MM, N=512, PSUM)
       mT/lT  += Wm_c.T/Wl_c.T @ hc  (4 MM accumulating, N=512)
  ACT: h0 = relu(hp0 + be0)  [PSUM->SBUF bf16]
       std = exp(0.5*lp + 0.5*bl)
  DVE: h1 = max(hp1 + be1, 0); mean = mp + bm; se = std*eps; z = mean+se
  GpS: logvar = lp + bl
DMA granules of 4096 rows (8 tiles) per stream, double buffered.
"""

import sys

sys.path.insert(0, "/opt/trn_rl_repo")

import numpy as np
import ml_dtypes

from contextlib import ExitStack

from concourse import bacc, mybir, tile
from concourse.alu_op_type import AluOpType
from concourse.bass_utils import run_bass_kernel_spmd

BF16_NP = ml_dtypes.bfloat16

N_CORES = 8
B = 524288
D_IN = 128
D_H = 256
D_OUT = 128
ROWS_PER_CORE = B // N_CORES  # 65536 (= columns per core in transposed space)
GCOLS = 4096  # DMA granule width (columns)
N_G = ROWS_PER_CORE // GCOLS  # 16
TCOLS = 512  # compute tile width (one PSUM bank)
N_T = GCOLS // TCOLS  # 8

F32 = mybir.dt.float32
BF16 = mybir.dt.bfloat16

Relu = mybir.ActivationFunctionType.Relu
Exp = mybir.ActivationFunctionType.Exp
Identity = mybir.ActivationFunctionType.Identity


def build_bass(cols_per_core=ROWS_PER_CORE):
    nc = bacc.Bacc("TRN2", target_bir_lowering=False, debug=False)
    n_g = cols_per_core // GCOLS

    xT = nc.declare_dram_parameter("xT", [D_IN, cols_per_core], BF16, isOutput=False)
    eT = nc.declare_dram_parameter("epsT", [D_OUT, cols_per_core], BF16, isOutput=False)
    We_ext = nc.declare_dram_parameter("W_emb", [D_IN, D_H], F32, isOutput=False)
    be_ext = nc.declare_dram_parameter("b_emb", [D_H], F32, isOutput=False)
    Wm_ext = nc.declare_dram_parameter("W_mean", [D_H, D_OUT], F32, isOutput=False)
    bm_ext = nc.declare_dram_parameter("b_mean", [D_OUT], F32, isOutput=False)
    Wl_ext = nc.declare_dram_parameter("W_logvar", [D_H, D_OUT], F32, isOutput=False)
    bl_ext = nc.declare_dram_parameter("b_logvar", [D_OUT], F32, isOutput=False)
    zT = nc.declare_dram_parameter("zT", [D_OUT, cols_per_core], BF16, isOutput=True)
    mT = nc.declare_dram_parameter("meanT", [D_OUT, cols_per_core], BF16, isOutput=True)
    lT = nc.declare_dram_parameter("lvT", [D_OUT, cols_per_core], BF16, isOutput=True)

    xv = xT.rearrange("d (g c) -> g d c", c=GCOLS)
    ev = eT.rearrange("d (g c) -> g d c", c=GCOLS)
    zv = zT.rearrange("d (g c) -> g d c", c=GCOLS)
    mv = mT.rearrange("d (g c) -> g d c", c=GCOLS)
    lv = lT.rearrange("d (g c) -> g d c", c=GCOLS)

    with tile.TileContext(nc) as tc, ExitStack() as ctx:
        const = ctx.enter_context(tc.tile_pool(name="const", bufs=1))
        xin = ctx.enter_context(tc.tile_pool(name="xin", bufs=2))
        ein = ctx.enter_context(tc.tile_pool(name="ein", bufs=2))
        outp = ctx.enter_context(tc.tile_pool(name="outp", bufs=2))
        hpool = ctx.enter_context(tc.tile_pool(name="hs", bufs=2))
        spool = ctx.enter_context(tc.tile_pool(name="sp", bufs=3))
        # Two persistent 4-bank mega tiles; tile t uses slot t%2 within
        # each. Slot rotation gives double-buffering via the framework's
        # per-region dependency tracking, while pair epilogue ops read
        # both slots of a pair as one FD=1024 op.
        psP = ctx.enter_context(tc.tile_pool(name="psP", bufs=1, space="PSUM"))

        # --- weights / biases (loaded once, bf16 via SWDGE cast-DMA) ---
        We_sb = const.tile([128, D_H], BF16)
        nc.gpsimd.dma_start(We_sb[:], We_ext[:])
        Wm_sb = const.tile([128, 2, D_OUT], BF16)
        Wl_sb = const.tile([128, 2, D_OUT], BF16)
        nc.gpsimd.dma_start(Wm_sb[:], Wm_ext.rearrange("(c p) d -> p c d", p=128))
        nc.gpsimd.dma_start(Wl_sb[:], Wl_ext.rearrange("(c p) d -> p c d", p=128))

        be_sb = const.tile([128, 2], F32)
        nc.sync.dma_start(be_sb[:], be_ext.rearrange("(c p) -> p c", p=128))
        bm_sb = const.tile([128, 1], F32)
        nc.sync.dma_start(bm_sb[:], bm_ext.rearrange("(p o) -> p o", o=1))
        bl_sb = const.tile([128, 1], F32)
        nc.sync.dma_start(bl_sb[:], bl_ext.rearrange("(p o) -> p o", o=1))
        blh_sb = const.tile([128, 1], F32)
        nc.vector.tensor_scalar_mul(blh_sb[:], bl_sb[:], 0.5)

        x_tiles = [None] * n_g
        e_tiles = [None] * n_g

        def load_granule(g):
            x_tiles[g] = xin.tile([128, GCOLS], BF16, tag="x", name="xg")
            nc.sync.dma_start(x_tiles[g][:], xv[g])
            e_tiles[g] = ein.tile([128, GCOLS], BF16, tag="e", name="eg")
            nc.sync.dma_start(e_tiles[g][:], ev[g])

        PCOLS = 2 * TCOLS  # pair width: every epilogue op runs at FD=1024
        # psH: [h0_even, h0_odd, h1_even, h1_odd]; psO: same layout for
        # [mp_even, mp_odd, lp_even, lp_odd]
        psH = psP.tile([128, 4, TCOLS], F32, tag="psH", name="psH")
        psO = psP.tile([128, 4, TCOLS], F32, tag="psO", name="psO")
        load_granule(0)
        for g in range(n_g):
            if g + 1 < n_g:
                load_granule(g + 1)
            xg = x_tiles[g]
            eg = e_tiles[g]
            x_tiles[g] = e_tiles[g] = None
            zg = outp.tile([128, GCOLS], BF16, tag="z")
            mg = outp.tile([128, GCOLS], BF16, tag="m")
            lg = outp.tile([128, GCOLS], BF16, tag="l")
            for p in range(GCOLS // PCOLS):
                psl = slice(p * PCOLS, (p + 1) * PCOLS)
                h0p_flat = psH[:, 0:2, :].rearrange("p a b -> p (a b)")
                h1p_flat = psH[:, 2:4, :].rearrange("p a b -> p (a b)")
                mp_flat = psO[:, 0:2, :].rearrange("p a b -> p (a b)")
                lp_flat = psO[:, 2:4, :].rearrange("p a b -> p (a b)")
                for tt in range(2):
                    sl = slice((2 * p + tt) * TCOLS, (2 * p + tt + 1) * TCOLS)
                    nc.tensor.matmul(
                        psH[:, tt, :], We_sb[:, 0:128], xg[:, sl], start=True, stop=True
                    )
                    nc.tensor.matmul(
                        psH[:, 2 + tt, :], We_sb[:, 128:256], xg[:, sl], start=True, stop=True
                    )

                h0 = hpool.tile([128, 2, TCOLS], BF16, tag="h0")
                h1 = hpool.tile([128, 2, TCOLS], BF16, tag="h1")
                # relu(h + be): chunk 0 on ACT, chunk 1 on DVE, pair-wide
                nc.scalar.activation(
                    h0[:].rearrange("p a b -> p (a b)"), h0p_flat, Relu,
                    bias=be_sb[:, 0:1],
                )
                nc.vector.tensor_scalar(
                    h1[:].rearrange("p a b -> p (a b)"), h1p_flat,
                    be_sb[:, 1:2], 0.0, AluOpType.add, AluOpType.max,
                )

                for tt in range(2):
                    nc.tensor.matmul(
                        psO[:, tt, :], Wm_sb[:, 0, :], h0[:, tt, :], start=True, stop=False
                    )
                    nc.tensor.matmul(
                        psO[:, tt, :], Wm_sb[:, 1, :], h1[:, tt, :], start=False, stop=True
                    )
                    nc.tensor.matmul(
                        psO[:, 2 + tt, :], Wl_sb[:, 0, :], h0[:, tt, :], start=True, stop=False
                    )
                    nc.tensor.matmul(
                        psO[:, 2 + tt, :], Wl_sb[:, 1, :], h1[:, tt, :], start=False, stop=True
                    )

                # pair-wide epilogue; mean/z leave the device WITHOUT b_mean
                # (host adds it); logvar gets b_logvar on ACT for free.
                std = spool.tile([128, PCOLS], BF16, tag="std")
                nc.scalar.activation(std[:], lp_flat, Exp, bias=blh_sb[:, 0:1], scale=0.5)
                nc.scalar.activation(lg[:, psl], lp_flat, Identity, bias=bl_sb[:, 0:1])
                nc.vector.tensor_copy(mg[:, psl], mp_flat)
                se = spool.tile([128, PCOLS], BF16, tag="se")
                nc.vector.tensor_mul(se[:], std[:], eg[:, psl])
                nc.gpsimd.tensor_add(zg[:, psl], mg[:, psl], se[:])

            nc.sync.dma_start(mv[g], mg[:])
            nc.sync.dma_start(lv[g], lg[:])
            nc.sync.dma_start(zv[g], zg[:])

    nc.finalize()
    return nc


_NC_CACHE = None


def _get_nc():
    global _NC_CACHE
    if _NC_CACHE is None:
        _NC_CACHE = build_bass()
    return _NC_CACHE


def _run(inputs, trace=False, **kw):
    nc = _get_nc()
    xs = np.asarray(inputs["x"], dtype=np.float32)
    es = np.asarray(inputs["eps"], dtype=np.float32)
    weights = {
        k: np.ascontiguousarray(np.asarray(inputs[k], dtype=np.float32))
        for k in ("W_emb", "b_emb", "W_mean", "b_mean", "W_logvar", "b_logvar")
    }
    in_maps = []
    for c in range(N_CORES):
        sl = slice(c * ROWS_PER_CORE, (c + 1) * ROWS_PER_CORE)
        in_maps.append(
            {
                "xT": xs[sl].T.astype(BF16_NP, order="C"),
                "epsT": es[sl].T.astype(BF16_NP, order="C"),
                **weights,
            }
        )
    res = run_bass_kernel_spmd(nc, in_maps, list(range(N_CORES)), trace=trace, **kw)
    z = np.empty((B, D_OUT), np.float32)
    mean = np.empty((B, D_OUT), np.float32)
    logvar = np.empty((B, D_OUT), np.float32)
    bm = weights["b_mean"]
    for c in range(N_CORES):
        sl = slice(c * ROWS_PER_CORE, (c + 1) * ROWS_PER_CORE)
        np.add(res.results[c]["zT"].T, bm, out=z[sl])
        np.add(res.results[c]["meanT"].T, bm, out=mean[sl])
        logvar[sl] = res.results[c]["lvT"].T
    return (z, mean, logvar), res


def kernel(**inputs):
    out, _ = _run(inputs, trace=False)
    return out


if __name__ == "__main__":
    rng = np.random.default_rng(0)
    demo = {
        "x": rng.standard_normal((B, D_IN), dtype=np.float32),
        "eps": rng.standard_normal((B, D_OUT), dtype=np.float32),
        "W_emb": rng.standard_normal((D_IN, D_H), dtype=np.float32) * 0.088,
        "b_emb": rng.standard_normal((D_H,), dtype=np.float32) * 0.05,
        "W_mean": rng.standard_normal((D_H, D_OUT), dtype=np.float32) * 0.06,
        "b_mean": rng.standard_normal((D_OUT,), dtype=np.float32) * 0.03,
        "W_logvar": rng.standard_normal((D_H, D_OUT), dtype=np.float32) * 0.06,
        "b_logvar": rng.standard_normal((D_OUT,), dtype=np.float32) * 0.03,
    }
    z, m, l = kernel(**demo)
    print("shapes", z.shape, m.shape, l.shape)


# revision 16
# speedup vs baseline: 1.7447x; 1.7447x over previous
"""GaussianMLP sampling kernel for 8 trn2 NeuronCores (pure data parallel).

reference:
    h      = relu(x @ W_emb + b_emb)        x:[B,128] W_emb:[128,256]
    mean   = h @ W_mean + b_mean            W_mean:[256,128]
    logvar = h @ W_logvar + b_logvar        W_logvar:[256,128]
    z      = mean + exp(0.5*logvar) * eps
    returns (z, mean, logvar)

Fully transposed dataflow: the host pre-transposes x/eps per core to
[d, rows] bf16, the device computes everything in [feature, row] space
(features on partitions), and the host transposes the three bf16
outputs back.  Wins vs. the row-major formulation:
  - no PE transposes and no K=1 bias-seed matmuls: every bias is
    per-partition, folded into ACT/DVE epilogue ops for free
  - all matmuls are bf16 with the small weight as the stationary
    operand and 512-row moving tiles (N=512)
  - all five HBM streams are bf16 (halves traffic vs f32) and fully
    contiguous: 1 MiB DMAs with 8 KiB per-partition runs

Per 512-row compute tile:
  PE : hT0/hT1 = We_c.T @ xT      (2 MM, N=512, PSUM)
       mT/lT  += Wm_c.T/Wl_c.T @ hc  (4 MM accumulating, N=512)
  ACT: h0 = relu(hp0 + be0)  [PSUM->SBUF bf16]
       std = exp(0.5*lp + 0.5*bl)
  DVE: h1 = max(hp1 + be1, 0); mean = mp + bm; se = std*eps; z = mean+se
  GpS: logvar = lp + bl
DMA granules of 4096 rows (8 tiles) per stream, double buffered.
"""

import sys

sys.path.insert(0, "/opt/trn_rl_repo")

import numpy as np
import ml_dtypes

from contextlib import ExitStack

from concourse import bacc, mybir, tile
from concourse.alu_op_type import AluOpType
from concourse.bass_utils import run_bass_kernel_spmd

BF16_NP = ml_dtypes.bfloat16

N_CORES = 8
B = 524288
D_IN = 128
D_H = 256
D_OUT = 128
ROWS_PER_CORE = B // N_CORES  # 65536 (= columns per core in transposed space)
GCOLS = 4096  # DMA granule width (columns)
N_G = ROWS_PER_CORE // GCOLS  # 16
TCOLS = 512  # compute tile width (one PSUM bank)
N_T = GCOLS // TCOLS  # 8

F32 = mybir.dt.float32
BF16 = mybir.dt.bfloat16

Relu = mybir.ActivationFunctionType.Relu
Exp = mybir.ActivationFunctionType.Exp
Identity = mybir.ActivationFunctionType.Identity


def build_bass(cols_per_core=ROWS_PER_CORE):
    nc = bacc.Bacc("TRN2", target_bir_lowering=False, debug=False)
    n_g = cols_per_core // GCOLS

    xT = nc.declare_dram_parameter("xT", [D_IN, cols_per_core], BF16, isOutput=False)
    eT = nc.declare_dram_parameter("epsT", [D_OUT, cols_per_core], BF16, isOutput=False)
    We_ext = nc.declare_dram_parameter("W_emb", [D_IN, D_H], F32, isOutput=False)
    be_ext = nc.declare_dram_parameter("b_emb", [D_H], F32, isOutput=False)
    Wm_ext = nc.declare_dram_parameter("W_mean", [D_H, D_OUT], F32, isOutput=False)
    bm_ext = nc.declare_dram_parameter("b_mean", [D_OUT], F32, isOutput=False)
    Wl_ext = nc.declare_dram_parameter("W_logvar", [D_H, D_OUT], F32, isOutput=False)
    bl_ext = nc.declare_dram_parameter("b_logvar", [D_OUT], F32, isOutput=False)
    zT = nc.declare_dram_parameter("zT", [D_OUT, cols_per_core], BF16, isOutput=True)
    mT = nc.declare_dram_parameter("meanT", [D_OUT, cols_per_core], BF16, isOutput=True)
    lT = nc.declare_dram_parameter("lvT", [D_OUT, cols_per_core], BF16, isOutput=True)

    xv = xT.rearrange("d (g c) -> g d c", c=GCOLS)
    ev = eT.rearrange("d (g c) -> g d c", c=GCOLS)
    zv = zT.rearrange("d (g c) -> g d c", c=GCOLS)
    mv = mT.rearrange("d (g c) -> g d c", c=GCOLS)
    lv = lT.rearrange("d (g c) -> g d c", c=GCOLS)

    with tile.TileContext(nc) as tc, ExitStack() as ctx:
        const = ctx.enter_context(tc.tile_pool(name="const", bufs=1))
        xin = ctx.enter_context(tc.tile_pool(name="xin", bufs=2))
        ein = ctx.enter_context(tc.tile_pool(name="ein", bufs=2))
        outp = ctx.enter_context(tc.tile_pool(name="outp", bufs=2))
        hpool = ctx.enter_context(tc.tile_pool(name="hs", bufs=2))
        spool = ctx.enter_context(tc.tile_pool(name="sp", bufs=3))
        psH = ctx.enter_context(tc.tile_pool(name="psH", bufs=2, space="PSUM"))
        psO = ctx.enter_context(tc.tile_pool(name="psO", bufs=2, space="PSUM"))

        # --- weights / biases (loaded once, bf16 via SWDGE cast-DMA) ---
        We_sb = const.tile([128, D_H], BF16)
        nc.gpsimd.dma_start(We_sb[:], We_ext[:])
        Wm_sb = const.tile([128, 2, D_OUT], BF16)
        Wl_sb = const.tile([128, 2, D_OUT], BF16)
        nc.gpsimd.dma_start(Wm_sb[:], Wm_ext.rearrange("(c p) d -> p c d", p=128))
        nc.gpsimd.dma_start(Wl_sb[:], Wl_ext.rearrange("(c p) d -> p c d", p=128))

        be_sb = const.tile([128, 2], F32)
        nc.sync.dma_start(be_sb[:], be_ext.rearrange("(c p) -> p c", p=128))
        bm_sb = const.tile([128, 1], F32)
        nc.sync.dma_start(bm_sb[:], bm_ext.rearrange("(p o) -> p o", o=1))
        bl_sb = const.tile([128, 1], F32)
        nc.sync.dma_start(bl_sb[:], bl_ext.rearrange("(p o) -> p o", o=1))
        blh_sb = const.tile([128, 1], F32)
        nc.vector.tensor_scalar_mul(blh_sb[:], bl_sb[:], 0.5)

        x_tiles = [None] * n_g
        e_tiles = [None] * n_g

        def load_granule(g):
            x_tiles[g] = xin.tile([128, GCOLS], BF16, tag="x", name="xg")
            nc.sync.dma_start(x_tiles[g][:], xv[g])
            e_tiles[g] = ein.tile([128, GCOLS], BF16, tag="e", name="eg")
            nc.sync.dma_start(e_tiles[g][:], ev[g])

        load_granule(0)
        for g in range(n_g):
            if g + 1 < n_g:
                load_granule(g + 1)
            xg = x_tiles[g]
            eg = e_tiles[g]
            x_tiles[g] = e_tiles[g] = None
            zg = outp.tile([128, GCOLS], BF16, tag="z")
            mg = outp.tile([128, GCOLS], BF16, tag="m")
            lg = outp.tile([128, GCOLS], BF16, tag="l")  # holds STD, not logvar
            for t in range(N_T):
                sl = slice(t * TCOLS, (t + 1) * TCOLS)
                hp0 = psH.tile([128, TCOLS], F32, tag="hp0")
                hp1 = psH.tile([128, TCOLS], F32, tag="hp1")
                nc.tensor.matmul(hp0[:], We_sb[:, 0:128], xg[:, sl], start=True, stop=True)
                nc.tensor.matmul(hp1[:], We_sb[:, 128:256], xg[:, sl], start=True, stop=True)

                h0 = hpool.tile([128, TCOLS], BF16, tag="h0")
                h1 = hpool.tile([128, TCOLS], BF16, tag="h1")
                # relu(h + be): chunk 0 on ACT, chunk 1 on DVE
                nc.scalar.activation(h0[:], hp0[:], Relu, bias=be_sb[:, 0:1])
                nc.vector.tensor_scalar(
                    h1[:], hp1[:], be_sb[:, 1:2], 0.0, AluOpType.add, AluOpType.max
                )

                mp = psO.tile([128, TCOLS], F32, tag="mp")
                lp = psO.tile([128, TCOLS], F32, tag="lp")
                nc.tensor.matmul(mp[:], Wm_sb[:, 0, :], h0[:], start=True, stop=False)
                nc.tensor.matmul(mp[:], Wm_sb[:, 1, :], h1[:], start=False, stop=True)
                nc.tensor.matmul(lp[:], Wl_sb[:, 0, :], h0[:], start=True, stop=False)
                nc.tensor.matmul(lp[:], Wl_sb[:, 1, :], h1[:], start=False, stop=True)

                # epilogue. Third output stream is std = exp(0.5*lv + 0.5*bl);
                # the host recovers logvar = 2*ln(std). mean/z leave the
                # device WITHOUT b_mean (host adds it).
                nc.scalar.activation(
                    lg[:, sl], lp[:], Exp, bias=blh_sb[:, 0:1], scale=0.5
                )
                nc.vector.tensor_copy(mg[:, sl], mp[:])
                se = spool.tile([128, TCOLS], BF16, tag="se")
                if t % 2 == 0:
                    nc.vector.tensor_mul(se[:], lg[:, sl], eg[:, sl])
                else:
                    nc.gpsimd.tensor_mul(se[:], lg[:, sl], eg[:, sl])
                nc.gpsimd.tensor_add(zg[:, sl], mg[:, sl], se[:])

            nc.sync.dma_start(mv[g], mg[:])
            nc.sync.dma_start(lv[g], lg[:])
            nc.sync.dma_start(zv[g], zg[:])

    nc.finalize()
    return nc


_NC_CACHE = None


def _get_nc():
    global _NC_CACHE
    if _NC_CACHE is None:
        _NC_CACHE = build_bass()
    return _NC_CACHE


def _run(inputs, trace=False, **kw):
    nc = _get_nc()
    xs = np.asarray(inputs["x"], dtype=np.float32)
    es = np.asarray(inputs["eps"], dtype=np.float32)
    weights = {
        k: np.ascontiguousarray(np.asarray(inputs[k], dtype=np.float32))
        for k in ("W_emb", "b_emb", "W_mean", "b_mean", "W_logvar", "b_logvar")
    }
    in_maps = []
    for c in range(N_CORES):
        sl = slice(c * ROWS_PER_CORE, (c + 1) * ROWS_PER_CORE)
        in_maps.append(
            {
                "xT": xs[sl].T.astype(BF16_NP, order="C"),
                "epsT": es[sl].T.astype(BF16_NP, order="C"),
                **weights,
            }
        )
    res = run_bass_kernel_spmd(nc, in_maps, list(range(N_CORES)), trace=trace, **kw)
    z = np.empty((B, D_OUT), np.float32)
    mean = np.empty((B, D_OUT), np.float32)
    logvar = np.empty((B, D_OUT), np.float32)
    bm = weights["b_mean"]
    for c in range(N_CORES):
        sl = slice(c * ROWS_PER_CORE, (c + 1) * ROWS_PER_CORE)
        np.add(res.results[c]["zT"].T, bm, out=z[sl])
        np.add(res.results[c]["meanT"].T, bm, out=mean[sl])
        # device ships std = exp(0.5*logvar); recover logvar = 2*ln(std)
        np.log(res.results[c]["lvT"].T.astype(np.float32), out=logvar[sl])
        logvar[sl] *= 2.0
    return (z, mean, logvar), res


def kernel(**inputs):
    out, _ = _run(inputs, trace=False)
    return out


if __name__ == "__main__":
    rng = np.random.default_rng(0)
    demo = {
        "x": rng.standard_normal((B, D_IN), dtype=np.float32),
        "eps": rng.standard_normal((B, D_OUT), dtype=np.float32),
        "W_emb": rng.standard_normal((D_IN, D_H), dtype=np.float32) * 0.088,
        "b_emb": rng.standard_normal((D_H,), dtype=np.float32) * 0.05,
        "W_mean": rng.standard_normal((D_H, D_OUT), dtype=np.float32) * 0.06,
        "b_mean": rng.standard_normal((D_OUT,), dtype=np.float32) * 0.03,
        "W_logvar": rng.standard_normal((D_H, D_OUT), dtype=np.float32) * 0.06,
        "b_logvar": rng.standard_normal((D_OUT,), dtype=np.float32) * 0.03,
    }
    z, m, l = kernel(**demo)
    print("shapes", z.shape, m.shape, l.shape)
